# revision 37
# baseline (speedup 1.0000x reference)
"""Trainium2 Bass kernel for nn_Attention_67370857005350 (v2, fp8).

Dense transformer block:
  q  = relu(pw_q  @ relu(bn(dwconv3x3(x))))            (2,512,64,64)
  kv = relu(pw_kv @ relu(bn(dwconv3x3_s2(features))))  (2,1024,32,32)
  out = relu(w_out @ softmax(q.k/8).v + b_out)         (2,256,64,64)

Sharding: spatial over query pixels - core c handles batch c//4, query
rows 16*(c%4) .. +16 (1024 q pixels); kv branch duplicated per batch.
No cross-core communication.

v2 changes vs the 147us baseline:
  * fp8(e4m3) operands + MatmulPerfMode.DoubleRow (0.5 cyc/row) for the
    dw convs (diagonal matmuls), pointwise convs, q.k and attn.v.
  * q.k contracts d=64 as [32,2]-k-tile DR; attn.v contracts kv pairs
    as [128,2] DR (the officially-shaped variant).
  * N=1024 matmuls (PSUM 2-bank outputs) to halve instruction count.
  * softmax numerators stored as expm1 in fp8: k is pre-scaled by
    0.125/sqrt(2) so exp(dots*0.125) = exp(sqrt2*x); poly chunks store
    e' = (x+sqrt2)*x ~= expm1(sqrt2 x) whose small magnitude survives
    fp8; exp chunks (ACT) store full e. A per-head correction
    S_h = sum_kv v over poly-routed chunks (tiny PE matmuls) restores
    u = e@v, and the ones-column row-sum gets the same fix.
  * exp work split across ACT (native exp) / DVE / Pool (quadratic
    poly), routed per chunk-pair to balance engine load.
  * input DMAs consolidated into a handful of descriptors on the two
    HWDGE rings, biggest-blockers first.
"""

import os
import numpy as np

import concourse.bass as bass
import concourse.tile as tile
from concourse import bacc, mybir
from concourse.bass_utils import run_bass_kernel_spmd

# ---- problem constants (hardcoded; must match setup_inputs) ----
B = 2
DIM = 256            # input channels
INNER = 512          # q/k/v channels
HEADS = 8
D = INNER // HEADS   # 64 head dim
HW_ = 64             # image H = W
KVHW = 32            # kv image H = W after stride-2
NKV = KVHW * KVHW    # 1024 kv pixels per batch
N_CORES = 8
CORES_PER_BATCH = N_CORES // B
ROWS = HW_ // CORES_PER_BATCH   # 16 q rows per core
NQ = ROWS * HW_                 # 1024 q pixels per core
EPS = 1e-5
LN128 = float(np.log(128.0))    # exp bias: store 128*exp(dots/8) in fp8

FP = mybir.dt.float32
BF = mybir.dt.bfloat16
F8 = mybir.dt.float8e4

AF = mybir.ActivationFunctionType
OP = mybir.AluOpType
PM = mybir.MatmulPerfMode

# feature flags (HW-probed)
QK8 = os.environ.get("K_QK8", "1") == "1"      # q.k as fp8 DR32
# N=1024 matmul outputs are illegal (PSUM bank crossing); keep the
# half-split path unless proven otherwise.
N1024 = os.environ.get("K_N1024", "0") == "1"
# per-(hp, j, P) routing of softmax-numerator units (one unit = the two
# [128,1024] pieces of an e tile): A=ACT exp (stores full e in fp8),
# D=DVE poly, G=Pool poly fed by a DVE copy (both store expm1 in fp8).
# Poly pairs lead (P=0,1) so the last-accumulated pair is the fast ACT
# one; counts tuned for ~equal engine busy time.
ROUTE = os.environ.get("K_ROUTE", "DGAA" * 4 + "DAAA" * 2 + "GAAA" * 2)


def _route(hp, j, P):
    return ROUTE[(hp * 2 + j) * 4 + P]


def _poly_pairs(h):
    hp, j = h // 2, h % 2
    return [P for P in range(4) if _route(hp, j, P) != "A"]


def _mm_n(nc, out, lhsT, rhs, start, stop, perf_mode=None, n1024=None,
          tile_position=None):
    """matmul, split into N<=512 pieces unless N1024."""
    n = rhs.shape[-1]
    use1024 = N1024 if n1024 is None else n1024
    if n <= 512 or use1024:
        nc.tensor.matmul(out, lhsT, rhs, start=start, stop=stop,
                         perf_mode=perf_mode, tile_position=tile_position)
    else:
        assert n == 1024
        for half in range(2):
            sl = slice(half * 512, (half + 1) * 512)
            nc.tensor.matmul(out[..., sl], lhsT, rhs[..., sl],
                             start=start, stop=stop, perf_mode=perf_mode,
                             tile_position=tile_position)


def build_graph():
    nc = bacc.Bacc("TRN2", target_bir_lowering=False, debug=False,
                   enable_asserts=False)

    def din(name, shape, dt):
        return nc.dram_tensor(name, shape, dt, kind="ExternalInput").ap()

    # per-core shards (host pads/permutes/folds; see _prep_shards).
    # The v-path weights (dgk, wv) stay bf16: their fp8 quantization
    # noise is correlated across kv and survives attention averaging
    # (measured 0.9-1.6e-2 rel err); the q/k-path fp8 noise is flattened
    # by the tiny softmax exponent range (~9e-5).
    xq_d = din("xq", [64, 4 * 18 * 66], F8)    # [64, ct2, s2, 18, 66]
    fs_d = din("fs", [128, 2 * 66 * 66], BF)   # [128, ct2, 66, 66]
    dgq_d = din("dgq", [64, 4 * 9 * 128], F8)  # [64, ct2, s2, 9, 128]
    dgk_d = din("dgk", [128, 2 * 9 * 128], BF)  # [128, ct2, 9, 128]
    w8_d = din("w8", [128, 2048], F8)          # wq8|wk8
    w16_d = din("w16", [128, 2048], BF)        # wo16|wv16
    cb_d = din("cb32", [128, 10], FP)          # tqb2|tkb2|bout2|c16|zero|ln128
    out_d = nc.dram_tensor("out", [128, 2 * NQ], FP,
                           kind="ExternalOutput").ap()



    with tile.TileContext(nc) as tc:
        with (
            tc.tile_pool(name="const", bufs=1) as cpool,
            tc.tile_pool(name="inbuf", bufs=1) as inpool,
            tc.tile_pool(name="act", bufs=1) as actpool,
            tc.tile_pool(name="exp", bufs=4) as epool,
            tc.tile_pool(name="tp", bufs=3) as tpool,
            tc.tile_pool(name="small", bufs=2) as spool,
            tc.tile_pool(name="ps", bufs=2, space="PSUM") as ps,
            tc.tile_pool(name="psu", bufs=2, space="PSUM") as psu,
        ):
            # ---------------- SBUF tiles ----------------
            xq = inpool.tile([64, 2, 2, 18, 66], F8)
            fs = inpool.tile([128, 2, 66, 66], BF)
            dgq = cpool.tile([64, 2, 2, 9, 128], F8)
            dgk = cpool.tile([128, 2, 9, 128], BF)
            w8 = cpool.tile([128, 2048], F8)
            wq8 = w8[:, 0:1024].rearrange("p (s m n) -> p s m n", s=2, m=4)
            wk8 = w8[:, 1024:2048].rearrange("p (s m n) -> p s m n", s=2, m=4)
            w16 = cpool.tile([128, 2048], BF)
            wo16 = w16[:, 0:1024].rearrange("p (k m n) -> p k m n", k=4, m=2)
            wv16 = w16[:, 1024:2048].rearrange("p (c n) -> p c n", c=2)
            cb = cpool.tile([128, 10], FP)

            # ---------------- input DMAs ----------------
            # scalar(ACT-HWDGE) ring: q-branch + weights, in gating order
            # sync(SP-HWDGE) ring: features image (kv branch), then output
            nc.scalar.dma_start(
                xq[:, :, :, :, :],
                xq_d.rearrange("p (c s a b) -> p c s a b", c=2, s=2, a=18))
            nc.scalar.dma_start(
                dgq[:, :, :, :, :],
                dgq_d.rearrange("p (c s t m) -> p c s t m", c=2, s=2, t=9))
            nc.scalar.dma_start(
                dgk[:, :, :, :],
                dgk_d.rearrange("p (c t m) -> p c t m", c=2, t=9))
            nc.scalar.dma_start(w8[:, :], w8_d)
            nc.scalar.dma_start(w16[:, :], w16_d)
            nc.scalar.dma_start(cb[:, :], cb_d)
            fs_r = fs_d.rearrange("p (c a b) -> p c a b", c=2, a=66)
            for ct in range(2):
                nc.sync.dma_start(fs[:, ct, :, :], fs_r[:, ct])

            tqb = cb[:, 0:2]    # per-ct dw-q bias columns
            tkb = cb[:, 2:4]
            boutc = cb[:, 4:6]  # per-mt to_out bias columns
            c16 = cb[:, 6:7]    # 16.0: poly (d+16)*d = 128*expm1(d/8)
            zc = cb[:, 7:8]     # zero column (AP scalar for relu max)
            ln128c = cb[:, 8:9]  # ln(128) exp bias

            # Pool's poly needs a full-width 16.0 tile (no TensorScalar
            # on Pool); ones8 holds 128.0 so S matches the 128x-scaled e.
            c16t = cpool.tile([128, 1024], BF)
            nc.gpsimd.memset(c16t[:, :], 16.0)
            ones8 = cpool.tile([128, 2, 32], F8)
            nc.gpsimd.memset(ones8[:, :, :], 128.0)

            tq = actpool.tile([128, 2, NQ], F8)     # ct = DR slot
            tkv = actpool.tile([128, 2, NKV], F8)   # for pw_k (DR)
            tkv16 = actpool.tile([128, 2, NKV], BF)  # for pw_v (bf16)
            # vt[p, P, h, s, m]: v^T staging, kv pair P slot s; m: v d
            # 0-63 | ones col 64 (row-sum trick) | zero pad to the legal
            # DR stationary width M=128 (M<128 non-pow2 crashes walrus)
            vt = actpool.tile([128, 4, HEADS, 2, 128], F8)
            nc.gpsimd.memset(vt[:, :, :, :, 64:65], 1.0)
            nc.gpsimd.memset(vt[:, :, :, :, 65:128], 0.0)
            if QK8:
                # q32/k32[p, hg, s, n]: head h at partitions 32*(h%4)..+32,
                # hg = h//4, s = d-half slot
                q32 = actpool.tile([128, 2, 2, NQ], F8)
                k32 = actpool.tile([128, 2, 2, NKV], F8)
            else:
                q_sb = actpool.tile([128, 4, NQ], BF)
                k_sb = actpool.tile([128, 4, NKV], BF)
            att = actpool.tile([128, 4, NQ], BF)
            s_sb = spool.tile([65, 8], FP, name="s_sb")
            osb = actpool.tile([128, 2, NQ], FP)

            # ---------------- depthwise convs on PE ----------------
            # q branch: fp8 DoubleRow diag matmuls ([64,2] channel tiles)
            # kv branch: bf16 diag matmuls (v-path precision)
            def dwq_conv(ct):
                acc = ps.tile([128, 1024], FP, tag="mm")
                for hf in range(2):
                    o = acc[:, hf * 512:(hf + 1) * 512]
                    for tap in range(9):
                        dy, dx = tap // 3, tap % 3
                        y0 = dy + hf * 8
                        rhs = xq[:, ct, :, y0:y0 + 8, dx:dx + 64]
                        nc.tensor.matmul(o, dgq[:, ct, :, tap, :], rhs,
                                         start=(tap == 0), stop=(tap == 8),
                                         perf_mode=PM.DoubleRow)
                nc.scalar.activation(tq[:, ct, :], acc[:, :], AF.Relu,
                                     bias=tqb[:, ct:ct + 1])

            def dwk_conv(ct):
                acc = ps.tile([128, 1024], FP, tag="mm")
                for hf in range(2):
                    o = acc[:, hf * 512:(hf + 1) * 512]
                    for tap in range(9):
                        dy, dx = tap // 3, tap % 3
                        y0 = dy + hf * 32
                        rhs = fs[:, ct, y0:y0 + 31:2, dx:dx + 63:2]
                        nc.tensor.matmul(o, dgk[:, ct, tap, :], rhs,
                                         start=(tap == 0), stop=(tap == 8))
                nc.scalar.activation(tkv16[:, ct, :], acc[:, :], AF.Relu,
                                     bias=tkb[:, ct:ct + 1])
                nc.scalar.activation(tkv[:, ct, :], acc[:, :], AF.Relu,
                                     bias=tkb[:, ct:ct + 1])

            for ct in range(2):
                dwq_conv(ct)
            for ct in range(2):
                dwk_conv(ct)

            # ---------------- pointwise convs (fp8 DR-128) -------------
            def pw_qk(wt, src, dst32, dst16, epi_eng):
                # column-parallel: out [ch 128, px] per mt; DR over in-ch
                for mt in range(4):
                    pq = ps.tile([128, 1024], FP, tag="mm")
                    _mm_n(nc, pq[:, :], wt[:, :, mt, :], src[:, :, :],
                          start=True, stop=True, perf_mode=PM.DoubleRow)
                    if QK8:
                        # psum rows [h0 d0-31 | h1 d0-31 | h0 d32+ | h1 d32+]
                        p0 = 64 * (mt % 2)
                        hg = mt // 2
                        for s in range(2):
                            epi_eng.activation(
                                dst32[p0:p0 + 64, hg, s, :],
                                pq[64 * s:64 * s + 64, :], AF.Relu)
                    else:
                        epi_eng.activation(dst16[:, mt, :], pq[:, :], AF.Relu)

            def pw_v():
                # row-parallel v^T (bf16): out [kv 128, vch 512] per chunk
                for i in range(4):
                    pv = ps.tile([128, 1024], FP, tag="mm")
                    for s in range(2):
                        c = 2 * i + s
                        for ct in range(2):
                            nc.tensor.matmul(
                                pv[:, 512 * s:512 * s + 512],
                                tkv16[:, ct, c * 128:(c + 1) * 128],
                                wv16[:, ct, :],
                                start=(ct == 0), stop=(ct == 1))
                    nc.vector.tensor_scalar(
                        vt[:, i, :, :, 0:64].rearrange("p h s d -> p s h d"),
                        pv[:, :].rearrange("p (s n) -> p s n", s=2),
                        zc[:, 0:1], None, op0=OP.max)

            if QK8:
                pw_qk(wq8, tq, q32, None, nc.scalar)
                pw_qk(wk8, tkv, k32, None, nc.scalar)
            else:
                pw_qk(wq8, tq, None, q_sb, nc.scalar)
                pw_qk(wk8, tkv, None, k_sb, nc.scalar)
            pw_v()

            # S_h = sum_kv v over poly-routed pairs (per-head correction
            # restoring the "+1" the expm1 chunks drop). Column h of one
            # [65, 1024] psum tile; copied to SBUF right away. Every head
            # must have >= 1 poly pair (the routing table guarantees it).
            nc.gpsimd.memset(s_sb[:, :], 0.0)
            if any(_poly_pairs(h) for h in range(HEADS)):
                s_ps = psu.tile([128, 1024], FP, tag="uR", name="s_ps")
                for h in range(HEADS):
                    pps = _poly_pairs(h)
                    for j, P in enumerate(pps):
                        nc.tensor.matmul(
                            s_ps[:, 32 * h:32 * h + 32], vt[:, P, h, :, :],
                            ones8[:, :, :],
                            start=(j == 0), stop=(j == len(pps) - 1),
                            perf_mode=PM.DoubleRow)
                    if pps:
                        nc.vector.tensor_copy(s_sb[:, h:h + 1],
                                              s_ps[0:65, 32 * h:32 * h + 1])

            # ---------------- attention ----------------
            def qk_mm(h, c, dp):
                if QK8:
                    p0 = 32 * (h % 4)
                    hg = h // 4
                    _mm_n(nc, dp[:, :],
                          k32[p0:p0 + 32, hg, :, c * 128:(c + 1) * 128],
                          q32[p0:p0 + 32, hg, :, :],
                          start=True, stop=True, perf_mode=PM.DoubleRow,
                          tile_position=(p0, 0))
                else:
                    p0 = 64 * (h % 2)
                    pt = h // 2
                    _mm_n(nc, dp[:, :],
                          k_sb[p0:p0 + 64, pt, c * 128:(c + 1) * 128],
                          q_sb[p0:p0 + 64, pt, :],
                          start=True, stop=True)

            def e_piece(route, dp, e, slot):
                # e8 pieces hold 128*softmax-numerator: A stores
                # 128*exp(d/8) via the ln(128) bias; D/G store
                # (d+16)*d = 128*expm1(d/8) + O(d^3) (Taylor-exact x^2)
                if route == "A":
                    nc.scalar.activation(e[:, slot, :], dp[:, :], AF.Exp,
                                         scale=0.125, bias=ln128c[:, 0:1])
                elif route == "D":
                    t = tpool.tile([128, 1024], FP, tag="t")
                    nc.vector.tensor_scalar(t[:, :], dp[:, :], c16[:, 0:1],
                                            None, op0=OP.add)
                    nc.vector.tensor_tensor(e[:, slot, :], t[:, :], dp[:, :],
                                            op=OP.mult)
                else:
                    # Pool cannot read PSUM: DVE stages x, Pool does the poly
                    xb = tpool.tile([128, 1024], BF, tag="xb")
                    nc.vector.tensor_copy(xb[:, :], dp[:, :])
                    t = tpool.tile([128, 1024], BF, tag="tb")
                    nc.gpsimd.tensor_tensor(t[:, :], xb[:, :], c16t[:, :],
                                            op=OP.add)
                    nc.gpsimd.tensor_tensor(e[:, slot, :], t[:, :], xb[:, :],
                                            op=OP.mult)

            def normalize(h, uR, last=False):
                kt = h // 2
                po = 64 * (h % 2)
                rrow = spool.tile([1, 1024], FP, tag="rrow",
                                  name=f"rrow_{h}")
                nc.vector.tensor_scalar(rrow[:, :], uR[64:65, :],
                                        s_sb[64:65, h:h + 1], None,
                                        op0=OP.add)
                invr = spool.tile([1, 1024], FP, tag="invr",
                                  name=f"invr_{h}")
                nc.vector.reciprocal_approx_fast(invr[:, :], rrow[:, :])
                invrb = spool.tile([64, 1024], FP, tag="invrb",
                                   name=f"invrb_{h}")
                nc.gpsimd.partition_broadcast(invrb[:, :], invr[:, :])
                nc.vector.scalar_tensor_tensor(
                    att[po:po + 64, kt, :], uR[0:64, :],
                    s_sb[0:64, h:h + 1], invrb[:, :],
                    op0=OP.add, op1=OP.mult)

            for hp in range(4):
                heads = (2 * hp, 2 * hp + 1)
                uRs = [psu.tile([128, 1024], FP, tag="uR",
                                name=f"uR_{hp}_{j}") for j in range(2)]
                for P in range(4):
                    for j, h in enumerate(heads):
                        e = epool.tile([128, 2, 1024], F8, tag="e",
                                       name=f"e_{hp}_{j}_{P}")
                        for s in range(2):
                            c = 2 * P + s
                            dp = ps.tile([128, 1024], FP, tag="mm",
                                         name=f"dp_{hp}_{j}_{c}")
                            qk_mm(h, c, dp)
                            e_piece(_route(hp, j, P), dp, e, s)
                        _mm_n(nc, uRs[j][:, :], vt[:, P, h, :, :],
                              e[:, :, :], start=(P == 0), stop=(P == 3),
                              perf_mode=PM.DoubleRow)
                for j, h in enumerate(heads):
                    normalize(h, uRs[j])

            # ---------------- to_out + epilogue ----------------
            for mt in range(2):
                oso = ps.tile([128, 1024], FP, tag="mm", name=f"oso_{mt}")
                for kt in range(4):
                    _mm_n(nc, oso[:, :], wo16[:, kt, mt, :], att[:, kt, :],
                          start=(kt == 0), stop=(kt == 3))
                eng = nc.scalar if mt == 0 else None
                if eng is not None:
                    eng.activation(osb[:, mt, :], oso[:, :], AF.Relu,
                                   bias=boutc[:, mt:mt + 1])
                else:
                    nc.vector.tensor_scalar(osb[:, mt, :], oso[:, :],
                                            boutc[:, mt:mt + 1], 0.0,
                                            op0=OP.add, op1=OP.max)
                nc.sync.dma_start(
                    out_d.rearrange("p (m n) -> p m n", m=2)[:, mt],
                    osb[:, mt, :])

    nc.compile()
    return nc


_NC_CACHE = {}


def _get_nc():
    key = (QK8, N1024, ROUTE)
    if key not in _NC_CACHE:
        _NC_CACHE[key] = build_graph()
    return _NC_CACHE[key]


def _prep_shards(inputs):
    """Host-side sharding/layout prep. Returns in_maps for the 8 cores."""
    import ml_dtypes
    f8 = ml_dtypes.float8_e4m3
    bf = ml_dtypes.bfloat16
    f32 = lambda a: np.ascontiguousarray(np.asarray(a, np.float32))

    x = f32(inputs["x"])
    features = f32(inputs["features"])

    # fold BN into depthwise weights/bias
    sq = f32(inputs["bnq_g"]) / np.sqrt(f32(inputs["bnq_v"]) + EPS)
    sk = f32(inputs["bnk_g"]) / np.sqrt(f32(inputs["bnk_v"]) + EPS)
    dwq = (f32(inputs["dw_q"])[:, 0] * sq[:, None, None]).reshape(DIM, 9)
    dwk = (f32(inputs["dw_kv"])[:, 0] * sk[:, None, None]).reshape(DIM, 9)
    tqb = f32(inputs["bnq_b"]) - f32(inputs["bnq_m"]) * sq
    tkb = f32(inputs["bnk_b"]) - f32(inputs["bnk_m"]) * sk

    pw_q = f32(inputs["pw_q"])[:, :, 0, 0]       # (512, 256)
    pw_kv = f32(inputs["pw_kv"])[:, :, 0, 0]     # (1024, 256)
    w_out = f32(inputs["w_out"])[:, :, 0, 0]     # (256, 512)
    b_out = f32(inputs["b_out"])

    # diagonal tap matrices. q branch: DR tiles [64, ct, s, t, m] fp8;
    # kv branch: full-diag [128, ct, t, m] bf16 (v-path precision)
    dgq = np.zeros((64, 2, 2, 9, 128), np.float32)
    p = np.arange(64)
    for ct in range(2):
        for s in range(2):
            ch = ct * 128 + s * 64 + p
            dgq[p, ct, s, :, s * 64 + p] = dwq[ch]
    dgq = np.ascontiguousarray(dgq.reshape(64, -1).astype(f8))
    dgk = np.zeros((128, 2, 9, 128), np.float32)
    p2 = np.arange(128)
    for ct in range(2):
        dgk[p2, ct, :, p2] = dwk[ct * 128 + p2]
    dgk = np.ascontiguousarray(dgk.reshape(128, -1).astype(bf))

    # pw weights, DR over in-ch: w[p, s, mt, m] = W[perm(mt,m), s*128+p]
    def perm_qk(mt, m):
        if not QK8:
            return mt * 128 + m
        j, r = divmod(m, 32)
        h = 2 * mt + (j % 2)
        d = (j // 2) * 32 + r
        return h * 64 + d

    wq8 = np.zeros((128, 2, 4, 128), np.float32)
    wk8 = np.zeros((128, 2, 4, 128), np.float32)
    for mt in range(4):
        for m in range(128):
            oc = perm_qk(mt, m)
            for s in range(2):
                wq8[:, s, mt, m] = pw_q[oc, s * 128:(s + 1) * 128]
                wk8[:, s, mt, m] = pw_kv[oc, s * 128:(s + 1) * 128]
    w8 = np.concatenate([wq8.reshape(128, -1), wk8.reshape(128, -1)],
                        axis=1).astype(f8)
    w8 = np.ascontiguousarray(w8)

    wo16 = np.zeros((128, 4, 2, 128), np.float32)
    for kt in range(4):
        for mt in range(2):
            wo16[:, kt, mt, :] = w_out[mt * 128:(mt + 1) * 128,
                                       kt * 128:(kt + 1) * 128].T
    wv16 = np.zeros((128, 2, 512), np.float32)
    for ct in range(2):
        wv16[:, ct, :] = pw_kv[INNER:, ct * 128:(ct + 1) * 128].T
    w16 = np.concatenate([wo16.reshape(128, -1), wv16.reshape(128, -1)],
                         axis=1).astype(bf)
    w16 = np.ascontiguousarray(w16)

    cb = np.zeros((128, 10), np.float32)
    cb[:, 0] = tqb[0:128]
    cb[:, 1] = tqb[128:256]
    cb[:, 2] = tkb[0:128]
    cb[:, 3] = tkb[128:256]
    cb[:, 4] = b_out[0:128]
    cb[:, 5] = b_out[128:256]
    cb[:, 6] = 16.0
    cb[:, 8] = LN128
    cb = np.ascontiguousarray(cb)

    # zero-padded images; xq in [64, ct, s, 18, 66] fp8 (DR channel
    # split), fs in [128, ct, 66, 66] bf16
    def img_split(img):  # img (DIM, 18, 66) padded slice
        h, w = img.shape[1], img.shape[2]
        o = np.zeros((64, 2, 2, h, w), np.float32)
        for ct in range(2):
            for s in range(2):
                o[:, ct, s] = img[ct * 128 + s * 64:ct * 128 + s * 64 + 64]
        return np.ascontiguousarray(o.reshape(64, -1).astype(f8))

    xpad = np.zeros((B, DIM, HW_ + 2, HW_ + 2), np.float32)
    xpad[:, :, 1:-1, 1:-1] = x
    fpad = np.zeros((B, DIM, HW_ + 2, HW_ + 2), np.float32)
    fpad[:, :, 1:-1, 1:-1] = features

    in_maps = []
    for c in range(N_CORES):
        b = c // CORES_PER_BATCH
        r0 = (c % CORES_PER_BATCH) * ROWS
        fs_b = np.ascontiguousarray(
            fpad[b].reshape(2, 128, 66, 66).transpose(1, 0, 2, 3)
            .reshape(128, -1).astype(bf))
        m = {
            "xq": img_split(xpad[b, :, r0:r0 + ROWS + 2, :]),
            "fs": fs_b,
            "dgq": dgq, "dgk": dgk, "w8": w8, "w16": w16, "cb32": cb,
        }
        in_maps.append(m)
    return in_maps


def kernel(**inputs):
    nc = _get_nc()
    in_maps = _prep_shards(inputs)
    trace = os.environ.get("KERNEL_TRACE", "0") == "1"
    res = run_bass_kernel_spmd(nc, in_maps, core_ids=list(range(N_CORES)),
                               trace=trace)
    if trace:
        kernel.last_exec_time_ns = res.exec_time_ns
        kernel.last_results = res
    out = np.zeros((B, DIM, HW_, HW_), np.float32)
    for c in range(N_CORES):
        b = c // CORES_PER_BATCH
        r0 = (c % CORES_PER_BATCH) * ROWS
        o = res.results[c]["out"].reshape(128, 2, ROWS, HW_)
        out[b, 0:128, r0:r0 + ROWS, :] = o[:, 0]
        out[b, 128:256, r0:r0 + ROWS, :] = o[:, 1]
    return out


if __name__ == "__main__":
    nc = build_graph()
    print("graph built + compiled OK")


# revision 42
# speedup vs baseline: 1.0480x; 1.0480x over previous
"""Trainium2 Bass kernel for nn_Attention_67370857005350 (v2, fp8).

Dense transformer block:
  q  = relu(pw_q  @ relu(bn(dwconv3x3(x))))            (2,512,64,64)
  kv = relu(pw_kv @ relu(bn(dwconv3x3_s2(features))))  (2,1024,32,32)
  out = relu(w_out @ softmax(q.k/8).v + b_out)         (2,256,64,64)

Sharding: spatial over query pixels - core c handles batch c//4, query
rows 16*(c%4) .. +16 (1024 q pixels); kv branch duplicated per batch.
No cross-core communication.

v2 changes vs the 147us baseline:
  * fp8(e4m3) operands + MatmulPerfMode.DoubleRow (0.5 cyc/row) for the
    dw convs (diagonal matmuls), pointwise convs, q.k and attn.v.
  * q.k contracts d=64 as [32,2]-k-tile DR; attn.v contracts kv pairs
    as [128,2] DR (the officially-shaped variant).
  * N=1024 matmuls (PSUM 2-bank outputs) to halve instruction count.
  * softmax numerators stored as expm1 in fp8: k is pre-scaled by
    0.125/sqrt(2) so exp(dots*0.125) = exp(sqrt2*x); poly chunks store
    e' = (x+sqrt2)*x ~= expm1(sqrt2 x) whose small magnitude survives
    fp8; exp chunks (ACT) store full e. A per-head correction
    S_h = sum_kv v over poly-routed chunks (tiny PE matmuls) restores
    u = e@v, and the ones-column row-sum gets the same fix.
  * exp work split across ACT (native exp) / DVE / Pool (quadratic
    poly), routed per chunk-pair to balance engine load.
  * input DMAs consolidated into a handful of descriptors on the two
    HWDGE rings, biggest-blockers first.
"""

import os
import numpy as np

import concourse.bass as bass
import concourse.tile as tile
from concourse import bacc, mybir
from concourse.bass_utils import run_bass_kernel_spmd

# ---- problem constants (hardcoded; must match setup_inputs) ----
B = 2
DIM = 256            # input channels
INNER = 512          # q/k/v channels
HEADS = 8
D = INNER // HEADS   # 64 head dim
HW_ = 64             # image H = W
KVHW = 32            # kv image H = W after stride-2
NKV = KVHW * KVHW    # 1024 kv pixels per batch
N_CORES = 8
CORES_PER_BATCH = N_CORES // B
ROWS = HW_ // CORES_PER_BATCH   # 16 q rows per core
NQ = ROWS * HW_                 # 1024 q pixels per core
EPS = 1e-5
LN128 = float(np.log(128.0))    # exp bias: store 128*exp(dots/8) in fp8

FP = mybir.dt.float32
BF = mybir.dt.bfloat16
F8 = mybir.dt.float8e4

AF = mybir.ActivationFunctionType
OP = mybir.AluOpType
PM = mybir.MatmulPerfMode

# feature flags (HW-probed)
QK8 = os.environ.get("K_QK8", "1") == "1"      # q.k as fp8 DR32
# N=1024 matmul outputs are illegal (PSUM bank crossing); keep the
# half-split path unless proven otherwise.
N1024 = os.environ.get("K_N1024", "0") == "1"
# per-piece routing of the 64 softmax-numerator pieces (one piece = one
# [128,1024] exp over a kv chunk): A=ACT exp (stores 128*e in fp8),
# D=DVE poly, G=Pool poly; D/G start with a DVE copy that frees the
# PSUM dots early and store 128*expm1. Chunk-level mixing makes the two
# pieces of a pv pair drain on different engines concurrently (PSUM
# cadence), with counts tuned for ~equal engine busy time.
ROUTE = os.environ.get("K_ROUTE", "AGDGAAAA" * 8)


def _route(hp, j, P, s):
    return ROUTE[(hp * 2 + j) * 8 + P * 2 + s]


def _poly_chunks(h):
    hp, j = h // 2, h % 2
    return [(P, s) for P in range(4) for s in range(2)
            if _route(hp, j, P, s) != "A"]


def _mm_n(nc, out, lhsT, rhs, start, stop, perf_mode=None, n1024=None,
          tile_position=None):
    """matmul, split into N<=512 pieces unless N1024."""
    n = rhs.shape[-1]
    use1024 = N1024 if n1024 is None else n1024
    if n <= 512 or use1024:
        nc.tensor.matmul(out, lhsT, rhs, start=start, stop=stop,
                         perf_mode=perf_mode, tile_position=tile_position)
    else:
        assert n == 1024
        for half in range(2):
            sl = slice(half * 512, (half + 1) * 512)
            nc.tensor.matmul(out[..., sl], lhsT, rhs[..., sl],
                             start=start, stop=stop, perf_mode=perf_mode,
                             tile_position=tile_position)


def build_graph():
    nc = bacc.Bacc("TRN2", target_bir_lowering=False, debug=False,
                   enable_asserts=False)

    def din(name, shape, dt):
        return nc.dram_tensor(name, shape, dt, kind="ExternalInput").ap()

    # per-core shards (host pads/permutes/folds; see _prep_shards).
    # The v-path weights (dgk, wv) stay bf16: their fp8 quantization
    # noise is correlated across kv and survives attention averaging
    # (measured 0.9-1.6e-2 rel err); the q/k-path fp8 noise is flattened
    # by the tiny softmax exponent range (~9e-5).
    xq_d = din("xq", [64, 4 * 18 * 66], F8)    # [64, ct2, s2, 18, 66]
    fs_d = din("fs", [128, 2 * 66 * 66], BF)   # [128, ct2, 66, 66]
    dgq_d = din("dgq", [64, 4 * 9 * 128], F8)  # [64, ct2, s2, 9, 128]
    dgk_d = din("dgk", [128, 2 * 9 * 128], BF)  # [128, ct2, 9, 128]
    w8_d = din("w8", [128, 2048], F8)          # wq8|wk8
    w16_d = din("w16", [128, 2048], BF)        # wo16|wv16
    cb_d = din("cb32", [128, 10], FP)          # tqb2|tkb2|bout2|c16|zero|ln128
    out_d = nc.dram_tensor("out", [128, 2 * NQ], FP,
                           kind="ExternalOutput").ap()



    with tile.TileContext(nc) as tc:
        with (
            tc.tile_pool(name="const", bufs=1) as cpool,
            tc.tile_pool(name="inbuf", bufs=1) as inpool,
            tc.tile_pool(name="act", bufs=1) as actpool,
            tc.tile_pool(name="exp", bufs=5) as epool,
            tc.tile_pool(name="tp", bufs=3) as tpool,
            tc.tile_pool(name="small", bufs=2) as spool,
            tc.tile_pool(name="ps", bufs=2, space="PSUM") as ps,
            tc.tile_pool(name="psu", bufs=2, space="PSUM") as psu,
        ):
            # ---------------- SBUF tiles ----------------
            xq = inpool.tile([64, 2, 2, 18, 66], F8)
            fs = inpool.tile([128, 2, 66, 66], BF)
            dgq = cpool.tile([64, 2, 2, 9, 128], F8)
            dgk = cpool.tile([128, 2, 9, 128], BF)
            w8 = cpool.tile([128, 2048], F8)
            wq8 = w8[:, 0:1024].rearrange("p (s m n) -> p s m n", s=2, m=4)
            wk8 = w8[:, 1024:2048].rearrange("p (s m n) -> p s m n", s=2, m=4)
            w16 = cpool.tile([128, 2048], BF)
            wo16 = w16[:, 0:1024].rearrange("p (k m n) -> p k m n", k=4, m=2)
            wv16 = w16[:, 1024:2048].rearrange("p (c n) -> p c n", c=2)
            cb = cpool.tile([128, 10], FP)

            # ---------------- input DMAs ----------------
            # scalar(ACT-HWDGE) ring: q-branch + weights, in gating order
            # sync(SP-HWDGE) ring: features image (kv branch), then output
            nc.scalar.dma_start(cb[:, :], cb_d)
            nc.scalar.dma_start(
                xq[:, :, :, :, :],
                xq_d.rearrange("p (c s a b) -> p c s a b", c=2, s=2, a=18))
            nc.scalar.dma_start(
                dgq[:, :, :, :, :],
                dgq_d.rearrange("p (c s t m) -> p c s t m", c=2, s=2, t=9))
            nc.scalar.dma_start(w8[:, :], w8_d)
            nc.sync.dma_start(
                dgk[:, :, :, :],
                dgk_d.rearrange("p (c t m) -> p c t m", c=2, t=9))
            fs_r = fs_d.rearrange("p (c a b) -> p c a b", c=2, a=66)
            for ct in range(2):
                nc.sync.dma_start(fs[:, ct, :, :], fs_r[:, ct])
            nc.sync.dma_start(w16[:, :], w16_d)

            tqb = cb[:, 0:2]    # per-ct dw-q bias columns
            tkb = cb[:, 2:4]
            boutc = cb[:, 4:6]  # per-mt to_out bias columns
            c16 = cb[:, 6:7]    # 16.0: poly (d+16)*d = 128*expm1(d/8)
            zc = cb[:, 7:8]     # zero column (AP scalar for relu max)
            ln128c = cb[:, 8:9]  # ln(128) exp bias

            # Pool's poly needs a full-width 16.0 tile (no TensorScalar
            # on Pool); ones8 holds 128.0 so S matches the 128x-scaled e.
            c16t = cpool.tile([128, 1024], BF)
            nc.gpsimd.memset(c16t[:, :], 16.0)
            ones8 = cpool.tile([128, 2, 32], F8)
            nc.gpsimd.memset(ones8[:, :, :], 128.0)

            tq = actpool.tile([128, 2, NQ], F8)     # ct = DR slot
            tkv = actpool.tile([128, 2, NKV], F8)   # for pw_k (DR)
            tkv16 = actpool.tile([128, 2, NKV], BF)  # for pw_v (bf16)
            # vt[p, P, h, s, m]: v^T staging, kv pair P slot s; m: v d
            # 0-63 | ones col 64 (row-sum trick) | zero pad to the legal
            # DR stationary width M=128 (M<128 non-pow2 crashes walrus)
            vt = actpool.tile([128, 4, HEADS, 2, 128], F8)
            nc.gpsimd.memset(vt[:, :, :, :, 64:65], 1.0)
            nc.gpsimd.memset(vt[:, :, :, :, 65:128], 0.0)
            if QK8:
                # q32/k32[p, hg, s, n]: head h at partitions 32*(h%4)..+32,
                # hg = h//4, s = d-half slot
                q32 = actpool.tile([128, 2, 2, NQ], F8)
                k32 = actpool.tile([128, 2, 2, NKV], F8)
            else:
                q_sb = actpool.tile([128, 4, NQ], BF)
                k_sb = actpool.tile([128, 4, NKV], BF)
            att = actpool.tile([128, 4, NQ], BF)
            s_sb = spool.tile([65, 8], FP, name="s_sb")
            osb = actpool.tile([128, 2, NQ], FP)

            # ---------------- depthwise convs on PE ----------------
            # q branch: fp8 DoubleRow diag matmuls ([64,2] channel tiles)
            # kv branch: bf16 diag matmuls (v-path precision)
            def dwq_conv(ct):
                acc = ps.tile([128, 1024], FP, tag="mm")
                for hf in range(2):
                    o = acc[:, hf * 512:(hf + 1) * 512]
                    for tap in range(9):
                        dy, dx = tap // 3, tap % 3
                        y0 = dy + hf * 8
                        rhs = xq[:, ct, :, y0:y0 + 8, dx:dx + 64]
                        nc.tensor.matmul(o, dgq[:, ct, :, tap, :], rhs,
                                         start=(tap == 0), stop=(tap == 8),
                                         perf_mode=PM.DoubleRow)
                nc.scalar.activation(tq[:, ct, :], acc[:, :], AF.Relu,
                                     bias=tqb[:, ct:ct + 1])

            def dwk_conv(ct):
                acc = ps.tile([128, 1024], FP, tag="mm")
                for hf in range(2):
                    o = acc[:, hf * 512:(hf + 1) * 512]
                    for tap in range(9):
                        dy, dx = tap // 3, tap % 3
                        y0 = dy + hf * 32
                        rhs = fs[:, ct, y0:y0 + 31:2, dx:dx + 63:2]
                        nc.tensor.matmul(o, dgk[:, ct, tap, :], rhs,
                                         start=(tap == 0), stop=(tap == 8))
                nc.scalar.activation(tkv16[:, ct, :], acc[:, :], AF.Relu,
                                     bias=tkb[:, ct:ct + 1])
                nc.scalar.activation(tkv[:, ct, :], acc[:, :], AF.Relu,
                                     bias=tkb[:, ct:ct + 1])

            for ct in range(2):
                dwq_conv(ct)
            for ct in range(2):
                dwk_conv(ct)

            # ---------------- pointwise convs (fp8 DR-128) -------------
            def pw_qk(wt, src, dst32, dst16, dve_epi=None):
                # column-parallel, out [ch 128, px] per mt; DR over in-ch.
                # With QK8 the host permutes the output channels so tile
                # mt covers exactly the (hg, s) = (mt//2, mt%2) slice of
                # the q32/k32 layout: one full-width epilogue per tile.
                for mt in range(4):
                    pq = ps.tile([128, 1024], FP, tag="mm")
                    _mm_n(nc, pq[:, :], wt[:, :, mt, :], src[:, :, :],
                          start=True, stop=True, perf_mode=PM.DoubleRow)
                    if QK8:
                        dst = dst32[:, mt // 2, mt % 2, :]
                    else:
                        dst = dst16[:, mt, :]
                    if dve_epi is not None:
                        nc.vector.tensor_scalar(dst, pq[:, :], zc[:, 0:1],
                                                None, op0=OP.max)
                    else:
                        nc.scalar.activation(dst, pq[:, :], AF.Relu)

            def pw_v():
                # row-parallel v^T (bf16): out [kv 128, vch 512] per chunk
                for i in range(4):
                    pv = ps.tile([128, 1024], FP, tag="mm")
                    for s in range(2):
                        c = 2 * i + s
                        for ct in range(2):
                            nc.tensor.matmul(
                                pv[:, 512 * s:512 * s + 512],
                                tkv16[:, ct, c * 128:(c + 1) * 128],
                                wv16[:, ct, :],
                                start=(ct == 0), stop=(ct == 1))
                    nc.vector.tensor_scalar(
                        vt[:, i, :, :, 0:64].rearrange("p h s d -> p s h d"),
                        pv[:, :].rearrange("p (s n) -> p s n", s=2),
                        zc[:, 0:1], None, op0=OP.max)

            if QK8:
                pw_qk(wq8, tq, q32, None)
                pw_qk(wk8, tkv, k32, None, dve_epi=True)
            else:
                pw_qk(wq8, tq, None, q_sb)
                pw_qk(wk8, tkv, None, k_sb)
            pw_v()

            # S_h = sum_kv v over poly-routed pairs (per-head correction
            # restoring the "+1" the expm1 chunks drop). Column h of one
            # [65, 1024] psum tile; copied to SBUF right away. Every head
            # must have >= 1 poly pair (the routing table guarantees it).
            nc.gpsimd.memset(s_sb[:, :], 0.0)
            if any(_poly_chunks(h) for h in range(HEADS)):
                s_ps = psu.tile([128, 1024], FP, tag="uR", name="s_ps")
                for h in range(HEADS):
                    pcs = _poly_chunks(h)
                    for j, (P, s) in enumerate(pcs):
                        nc.tensor.matmul(
                            s_ps[:, 32 * h:32 * h + 32],
                            vt[:, P, h, s, :], ones8[:, 0, :],
                            start=(j == 0), stop=(j == len(pcs) - 1))
                    if pcs:
                        nc.vector.tensor_copy(s_sb[:, h:h + 1],
                                              s_ps[0:65, 32 * h:32 * h + 1])

            # ---------------- attention ----------------
            def qk_mm(h, c, dp):
                if QK8:
                    p0 = 32 * (h % 4)
                    hg = h // 4
                    _mm_n(nc, dp[:, :],
                          k32[p0:p0 + 32, hg, :, c * 128:(c + 1) * 128],
                          q32[p0:p0 + 32, hg, :, :],
                          start=True, stop=True, perf_mode=PM.DoubleRow,
                          tile_position=(p0, 0))
                else:
                    p0 = 64 * (h % 2)
                    pt = h // 2
                    _mm_n(nc, dp[:, :],
                          k_sb[p0:p0 + 64, pt, c * 128:(c + 1) * 128],
                          q_sb[p0:p0 + 64, pt, :],
                          start=True, stop=True)

            def e_piece(route, dp, e, slot):
                # e8 pieces hold 128*softmax-numerator: A stores
                # 128*exp(d/8) via the ln(128) bias; D/G copy the dots
                # to SBUF first (frees the PSUM bank after one op) and
                # store (d+16)*d = 128*expm1(d/8) + O(d^3)
                if route == "A":
                    nc.scalar.activation(e[:, slot, :], dp[:, :], AF.Exp,
                                         scale=0.125, bias=ln128c[:, 0:1])
                elif route == "D":
                    xb = tpool.tile([128, 1024], BF, tag="xb")
                    nc.vector.tensor_copy(xb[:, :], dp[:, :])
                    nc.vector.scalar_tensor_tensor(
                        e[:, slot, :], xb[:, :], c16[:, 0:1], xb[:, :],
                        op0=OP.add, op1=OP.mult)
                else:
                    # Pool cannot read PSUM: DVE stages x, Pool does the poly
                    xb = tpool.tile([128, 1024], BF, tag="xb")
                    nc.vector.tensor_copy(xb[:, :], dp[:, :])
                    t = tpool.tile([128, 1024], BF, tag="tb")
                    nc.gpsimd.tensor_tensor(t[:, :], xb[:, :], c16t[:, :],
                                            op=OP.add)
                    nc.gpsimd.tensor_tensor(e[:, slot, :], t[:, :], xb[:, :],
                                            op=OP.mult)

            def normalize(h, uR, last=False):
                kt = h // 2
                po = 64 * (h % 2)
                rrow = spool.tile([1, 1024], FP, tag="rrow",
                                  name=f"rrow_{h}")
                # row sums are strictly positive, so Relu == identity
                # (Copy rejects AP biases)
                nc.scalar.activation(rrow[:, :], uR[64:65, :], AF.Relu,
                                     bias=s_sb[64:65, h:h + 1])
                invr = spool.tile([1, 1024], FP, tag="invr",
                                  name=f"invr_{h}")
                nc.vector.reciprocal_approx_fast(invr[:, :], rrow[:, :])
                invrb = spool.tile([64, 1024], FP, tag="invrb",
                                   name=f"invrb_{h}")
                nc.gpsimd.partition_broadcast(invrb[:, :], invr[:, :])
                nc.vector.scalar_tensor_tensor(
                    att[po:po + 64, kt, :], uR[0:64, :],
                    s_sb[0:64, h:h + 1], invrb[:, :],
                    op0=OP.add, op1=OP.mult)

            def pv_mm(uR, P, h, e):
                _mm_n(nc, uR[:, :], vt[:, P, h, :, :],
                      e[:, :, :], start=(P == 0), stop=(P == 3),
                      perf_mode=PM.DoubleRow)

            for hp in range(4):
                heads = (2 * hp, 2 * hp + 1)
                uRs = [psu.tile([128, 1024], FP, tag="uR",
                                name=f"uR_{hp}_{j}") for j in range(2)]
                epend = {}
                for P in range(4):
                    for j, h in enumerate(heads):
                        e = epool.tile([128, 2, 1024], F8, tag="e",
                                       name=f"e_{hp}_{j}_{P}")
                        epend[(P, j)] = e
                        for s in range(2):
                            c = 2 * P + s
                            dp = ps.tile([128, 1024], FP, tag="mm",
                                         name=f"dp_{hp}_{j}_{c}")
                            qk_mm(h, c, dp)
                            e_piece(_route(hp, j, P, s), dp, e, s)
                        # pv lags one pair so the PE queue never blocks
                        # on an e tile still being produced
                        if P >= 1:
                            pv_mm(uRs[j], P - 1, h, epend.pop((P - 1, j)))
                for j, h in enumerate(heads):
                    pv_mm(uRs[j], 3, h, epend.pop((3, j)))
                    normalize(h, uRs[j])

            # ---------------- to_out + epilogue ----------------
            for mt in range(2):
                oso = ps.tile([128, 1024], FP, tag="mm", name=f"oso_{mt}")
                for kt in range(4):
                    _mm_n(nc, oso[:, :], wo16[:, kt, mt, :], att[:, kt, :],
                          start=(kt == 0), stop=(kt == 3))
                eng = nc.scalar if mt == 0 else None
                if eng is not None:
                    eng.activation(osb[:, mt, :], oso[:, :], AF.Relu,
                                   bias=boutc[:, mt:mt + 1])
                else:
                    nc.vector.tensor_scalar(osb[:, mt, :], oso[:, :],
                                            boutc[:, mt:mt + 1], 0.0,
                                            op0=OP.add, op1=OP.max)
                nc.sync.dma_start(
                    out_d.rearrange("p (m n) -> p m n", m=2)[:, mt],
                    osb[:, mt, :])

    nc.compile()
    return nc


_NC_CACHE = {}


def _get_nc():
    key = (QK8, N1024, ROUTE)
    if key not in _NC_CACHE:
        _NC_CACHE[key] = build_graph()
    return _NC_CACHE[key]


def _prep_shards(inputs):
    """Host-side sharding/layout prep. Returns in_maps for the 8 cores."""
    import ml_dtypes
    f8 = ml_dtypes.float8_e4m3
    bf = ml_dtypes.bfloat16
    f32 = lambda a: np.ascontiguousarray(np.asarray(a, np.float32))

    x = f32(inputs["x"])
    features = f32(inputs["features"])

    # fold BN into depthwise weights/bias
    sq = f32(inputs["bnq_g"]) / np.sqrt(f32(inputs["bnq_v"]) + EPS)
    sk = f32(inputs["bnk_g"]) / np.sqrt(f32(inputs["bnk_v"]) + EPS)
    dwq = (f32(inputs["dw_q"])[:, 0] * sq[:, None, None]).reshape(DIM, 9)
    dwk = (f32(inputs["dw_kv"])[:, 0] * sk[:, None, None]).reshape(DIM, 9)
    tqb = f32(inputs["bnq_b"]) - f32(inputs["bnq_m"]) * sq
    tkb = f32(inputs["bnk_b"]) - f32(inputs["bnk_m"]) * sk

    pw_q = f32(inputs["pw_q"])[:, :, 0, 0]       # (512, 256)
    pw_kv = f32(inputs["pw_kv"])[:, :, 0, 0]     # (1024, 256)
    w_out = f32(inputs["w_out"])[:, :, 0, 0]     # (256, 512)
    b_out = f32(inputs["b_out"])

    # diagonal tap matrices. q branch: DR tiles [64, ct, s, t, m] fp8;
    # kv branch: full-diag [128, ct, t, m] bf16 (v-path precision)
    dgq = np.zeros((64, 2, 2, 9, 128), np.float32)
    p = np.arange(64)
    for ct in range(2):
        for s in range(2):
            ch = ct * 128 + s * 64 + p
            dgq[p, ct, s, :, s * 64 + p] = dwq[ch]
    dgq = np.ascontiguousarray(dgq.reshape(64, -1).astype(f8))
    dgk = np.zeros((128, 2, 9, 128), np.float32)
    p2 = np.arange(128)
    for ct in range(2):
        dgk[p2, ct, :, p2] = dwk[ct * 128 + p2]
    dgk = np.ascontiguousarray(dgk.reshape(128, -1).astype(bf))

    # pw weights, DR over in-ch: w[p, s, mt, m] = W[perm(mt,m), s*128+p]
    def perm_qk(mt, m):
        # q32/k32 partition p holds head 4*hg + p//32, d-low p%32, with
        # d-half s in the free dim; tile mt = (hg, s) = (mt//2, mt%2)
        if not QK8:
            return mt * 128 + m
        h = 4 * (mt // 2) + m // 32
        d = (mt % 2) * 32 + m % 32
        return h * 64 + d

    wq8 = np.zeros((128, 2, 4, 128), np.float32)
    wk8 = np.zeros((128, 2, 4, 128), np.float32)
    for mt in range(4):
        for m in range(128):
            oc = perm_qk(mt, m)
            for s in range(2):
                wq8[:, s, mt, m] = pw_q[oc, s * 128:(s + 1) * 128]
                wk8[:, s, mt, m] = pw_kv[oc, s * 128:(s + 1) * 128]
    w8 = np.concatenate([wq8.reshape(128, -1), wk8.reshape(128, -1)],
                        axis=1).astype(f8)
    w8 = np.ascontiguousarray(w8)

    wo16 = np.zeros((128, 4, 2, 128), np.float32)
    for kt in range(4):
        for mt in range(2):
            wo16[:, kt, mt, :] = w_out[mt * 128:(mt + 1) * 128,
                                       kt * 128:(kt + 1) * 128].T
    wv16 = np.zeros((128, 2, 512), np.float32)
    for ct in range(2):
        wv16[:, ct, :] = pw_kv[INNER:, ct * 128:(ct + 1) * 128].T
    w16 = np.concatenate([wo16.reshape(128, -1), wv16.reshape(128, -1)],
                         axis=1).astype(bf)
    w16 = np.ascontiguousarray(w16)

    cb = np.zeros((128, 10), np.float32)
    cb[:, 0] = tqb[0:128]
    cb[:, 1] = tqb[128:256]
    cb[:, 2] = tkb[0:128]
    cb[:, 3] = tkb[128:256]
    cb[:, 4] = b_out[0:128]
    cb[:, 5] = b_out[128:256]
    cb[:, 6] = 16.0
    cb[:, 8] = LN128
    cb = np.ascontiguousarray(cb)

    # zero-padded images; xq in [64, ct, s, 18, 66] fp8 (DR channel
    # split), fs in [128, ct, 66, 66] bf16
    def img_split(img):  # img (DIM, 18, 66) padded slice
        h, w = img.shape[1], img.shape[2]
        o = np.zeros((64, 2, 2, h, w), np.float32)
        for ct in range(2):
            for s in range(2):
                o[:, ct, s] = img[ct * 128 + s * 64:ct * 128 + s * 64 + 64]
        return np.ascontiguousarray(o.reshape(64, -1).astype(f8))

    xpad = np.zeros((B, DIM, HW_ + 2, HW_ + 2), np.float32)
    xpad[:, :, 1:-1, 1:-1] = x
    fpad = np.zeros((B, DIM, HW_ + 2, HW_ + 2), np.float32)
    fpad[:, :, 1:-1, 1:-1] = features

    in_maps = []
    for c in range(N_CORES):
        b = c // CORES_PER_BATCH
        r0 = (c % CORES_PER_BATCH) * ROWS
        fs_b = np.ascontiguousarray(
            fpad[b].reshape(2, 128, 66, 66).transpose(1, 0, 2, 3)
            .reshape(128, -1).astype(bf))
        m = {
            "xq": img_split(xpad[b, :, r0:r0 + ROWS + 2, :]),
            "fs": fs_b,
            "dgq": dgq, "dgk": dgk, "w8": w8, "w16": w16, "cb32": cb,
        }
        in_maps.append(m)
    return in_maps


def kernel(**inputs):
    nc = _get_nc()
    in_maps = _prep_shards(inputs)
    trace = os.environ.get("KERNEL_TRACE", "0") == "1"
    res = run_bass_kernel_spmd(nc, in_maps, core_ids=list(range(N_CORES)),
                               trace=trace)
    if trace:
        kernel.last_exec_time_ns = res.exec_time_ns
        kernel.last_results = res
    out = np.zeros((B, DIM, HW_, HW_), np.float32)
    for c in range(N_CORES):
        b = c // CORES_PER_BATCH
        r0 = (c % CORES_PER_BATCH) * ROWS
        o = res.results[c]["out"].reshape(128, 2, ROWS, HW_)
        out[b, 0:128, r0:r0 + ROWS, :] = o[:, 0]
        out[b, 128:256, r0:r0 + ROWS, :] = o[:, 1]
    return out


if __name__ == "__main__":
    nc = build_graph()
    print("graph built + compiled OK")
